# revision 4
# baseline (speedup 1.0000x reference)
"""Trainium2 Bass kernel for nn_ExtraPositionPromptSABottleneck.

Data-parallel over batch B=8 across 8 NeuronCores; each core computes one
batch element's full bottleneck block:

  x1 = silu(bn1(cv1 @ x))            [C=256, N=4096]
  q/k/e = proj(x1); v^T materialized directly via matmul
  S^T[m,n] = k^T q + (rel-pos term)  computed in transposed layout so that
             softmax-exp needs no max subtraction (scores are |s|<40) and
             the output matmul out = v @ attn^T needs no 4096^2 transpose.
  rel-pos:   pos^T e = H^T A + W^T B with A = rh^T e, B = rw^T e ([64,4096])
             and H/W constant one-hots -> packed as a third 128-row K-chunk
             [A;B] x [H;W] of the S matmul.
  softmax:   k_b/e_b biases drop out (softmax row-shift invariance);
             row sums via ones-vector matmul; normalization deferred to the
             PSUM->SBUF evacuation of the output accumulator.
  y = silu(bn2(cv2 @ out_norm)) + x

All matmuls run in float32r (single-pass fp32 at bf16 rate); the one-hot
chunk runs in bf16 (exact 0/1 values).
"""

import os
import numpy as np
import ml_dtypes

import concourse.bass as bass
import concourse.tile as tile
from concourse import bacc, mybir
from concourse.bass_utils import run_bass_kernel_spmd

f32 = mybir.dt.float32
f32r = mybir.dt.float32r
bf16 = mybir.dt.bfloat16
AF = mybir.ActivationFunctionType

B, DIMS, SIZE = 8, 512, 64
C = DIMS // 2              # 256
N = SIZE * SIZE            # 4096
NBLK = 512                 # column block (one PSUM bank of fp32)
NNB = N // NBLK            # 8 n blocks
MB = N // 128              # 32 m blocks
EPS = 1e-5


def build_nc():
    nc = bacc.Bacc("TRN2", target_bir_lowering=False, debug=False)

    x_d = nc.dram_tensor("x", [DIMS, N], f32, kind="ExternalInput")
    cv1_d = nc.dram_tensor("cv1_lhsT", [DIMS, C], f32, kind="ExternalInput")
    b1_d = nc.dram_tensor("b1", [128, 2], f32, kind="ExternalInput")
    qw_d = nc.dram_tensor("q_lhsT", [C, C], f32, kind="ExternalInput")
    qb_d = nc.dram_tensor("q_bias", [128, 2], f32, kind="ExternalInput")
    kw_d = nc.dram_tensor("k_lhsT", [C, C], f32, kind="ExternalInput")
    ew_d = nc.dram_tensor("e_lhsT", [C, C], f32, kind="ExternalInput")
    vw_d = nc.dram_tensor("v_rhs", [C, C], f32, kind="ExternalInput")
    vb_d = nc.dram_tensor("v_bias_row", [1, C], f32, kind="ExternalInput")
    rh_d = nc.dram_tensor("rh", [C, SIZE], f32, kind="ExternalInput")
    rw_d = nc.dram_tensor("rw", [C, SIZE], f32, kind="ExternalInput")
    oh_d = nc.dram_tensor("onehot", [128, N], bf16, kind="ExternalInput")
    cv2_d = nc.dram_tensor("cv2_lhsT", [C, DIMS], f32, kind="ExternalInput")
    b2_d = nc.dram_tensor("b2", [128, 4], f32, kind="ExternalInput")
    ones_d = nc.dram_tensor("ones_col", [128, 1], f32, kind="ExternalInput")
    ones1_d = nc.dram_tensor("ones_row", [1, 128], f32, kind="ExternalInput")
    y_d = nc.dram_tensor("y", [DIMS, N], f32, kind="ExternalOutput")

    with tile.TileContext(nc) as tc:
        with (
            tc.tile_pool(name="wp", bufs=1) as wp,
            tc.tile_pool(name="qp", bufs=1) as qp_pool,
            tc.tile_pool(name="kep", bufs=1) as ke_pool,
            tc.tile_pool(name="vtp", bufs=1) as vt_pool,
            tc.tile_pool(name="bigbuf", bufs=2) as bigbuf,
            tc.tile_pool(name="xpanels", bufs=2) as xp_pool,
            tc.tile_pool(name="work", bufs=6) as work,
            tc.tile_pool(name="small", bufs=2) as small,
        ):
            # ---- weights / constants ----
            cv1_t = wp.tile([128, 4, C], f32r, tag="cv1_t")
            nc.gpsimd.dma_start(cv1_t[:], cv1_d.rearrange("(k p) m -> p k m", p=128))
            qw_t = wp.tile([128, 2, C], f32r, tag="qw_t")
            nc.gpsimd.dma_start(qw_t[:], qw_d.rearrange("(k p) m -> p k m", p=128))
            kw_t = wp.tile([128, 2, C], f32r, tag="kw_t")
            nc.gpsimd.dma_start(kw_t[:], kw_d.rearrange("(k p) m -> p k m", p=128))
            ew_t = wp.tile([128, 2, C], f32r, tag="ew_t")
            nc.gpsimd.dma_start(ew_t[:], ew_d.rearrange("(k p) m -> p k m", p=128))
            vw_t = wp.tile([128, 2, C], f32r, tag="vw_t")
            nc.gpsimd.dma_start(vw_t[:], vw_d.rearrange("(k p) m -> p k m", p=128))
            vb_t = wp.tile([1, C], f32r, tag="vb_t")
            nc.gpsimd.dma_start(vb_t[:], vb_d[:])
            rh_t = wp.tile([128, 2, SIZE], f32r, tag="rh_t")
            nc.gpsimd.dma_start(rh_t[:], rh_d.rearrange("(k p) m -> p k m", p=128))
            rw_t = wp.tile([128, 2, SIZE], f32r, tag="rw_t")
            nc.gpsimd.dma_start(rw_t[:], rw_d.rearrange("(k p) m -> p k m", p=128))
            cv2_t = wp.tile([128, 2, DIMS], f32r, tag="cv2_t")
            nc.gpsimd.dma_start(cv2_t[:], cv2_d.rearrange("(k p) m -> p k m", p=128))
            ones_t = wp.tile([128, 1], f32r, tag="ones_t")
            nc.gpsimd.dma_start(ones_t[:], ones_d[:])
            ones1_t = wp.tile([1, 128], f32r, tag="ones1_t")
            nc.gpsimd.dma_start(ones1_t[:], ones1_d[:])
            b1_t = wp.tile([128, 2], f32, tag="b1_t")
            nc.sync.dma_start(b1_t[:], b1_d[:])
            qb_t = wp.tile([128, 2], f32, tag="qb_t")
            nc.sync.dma_start(qb_t[:], qb_d[:])
            b2_t = wp.tile([128, 4], f32, tag="b2_t")
            nc.sync.dma_start(b2_t[:], b2_d[:])
            oh_t = wp.tile([128, N], bf16, tag="oh_t")
            nc.sync.dma_start(oh_t[:], oh_d[:])

            # ---- persistent big tensors ----
            qp_t = [qp_pool.tile([128, N], f32r, tag=f"qp{c}", name=f"qp{c}")
                    for c in range(2)]
            ke_t = [ke_pool.tile([128, N], f32r, tag=f"ke{c}", name=f"ke{c}")
                    for c in range(2)]
            ab_t = ke_pool.tile([128, N], bf16, tag="ab")
            vt_t = vt_pool.tile([128, MB * C], f32r, tag="vt")
            x1_t = [bigbuf.tile([128, N], f32r, tag="big", name=f"x1_{c}")
                    for c in range(2)]

            # =========== Phase A+B: x -> x1 -> q,k,e,AB,vT (per n-block) ======
            with tc.tile_pool(name="psAB", bufs=6, space="PSUM") as psAB:
                for nb in range(NNB):
                    ns = bass.ts(nb, NBLK)
                    xp = []
                    for kc in range(4):
                        xt = xp_pool.tile([128, NBLK], f32r, tag=f"x{kc}",
                                          name=f"xp{kc}_{nb}")
                        nc.gpsimd.dma_start(xt[:], x_d[bass.ts(kc, 128), ns])
                        xp.append(xt)
                    # x1 = silu(cv1' @ x + b1')
                    for cb in range(2):
                        ps = psAB.tile([128, NBLK], f32, tag="ps", name=f"x1ps{nb}_{cb}")
                        for kc in range(4):
                            nc.tensor.matmul(ps[:], cv1_t[:, kc, bass.ts(cb, 128)],
                                             xp[kc][:], start=(kc == 0), stop=(kc == 3))
                        nc.scalar.activation(x1_t[cb][:, ns], ps[:], AF.Silu,
                                             bias=b1_t[:, cb:cb + 1])
                    # q (bias q_b), k (no bias)
                    for cb in range(2):
                        ps = psAB.tile([128, NBLK], f32, tag="ps", name=f"qps{nb}_{cb}")
                        for kc in range(2):
                            nc.tensor.matmul(ps[:], qw_t[:, kc, bass.ts(cb, 128)],
                                             x1_t[kc][:, ns], start=(kc == 0),
                                             stop=(kc == 1))
                        nc.scalar.activation(qp_t[cb][:, ns], ps[:], AF.Identity,
                                             bias=qb_t[:, cb:cb + 1])
                    for cb in range(2):
                        ps = psAB.tile([128, NBLK], f32, tag="ps", name=f"kps{nb}_{cb}")
                        for kc in range(2):
                            nc.tensor.matmul(ps[:], kw_t[:, kc, bass.ts(cb, 128)],
                                             x1_t[kc][:, ns], start=(kc == 0),
                                             stop=(kc == 1))
                        nc.vector.tensor_copy(ke_t[cb][:, ns], ps[:])
                    # e panels (consumed immediately by A/B matmuls)
                    ep = []
                    for cb in range(2):
                        ps = psAB.tile([128, NBLK], f32, tag="ps", name=f"eps{nb}_{cb}")
                        for kc in range(2):
                            nc.tensor.matmul(ps[:], ew_t[:, kc, bass.ts(cb, 128)],
                                             x1_t[kc][:, ns], start=(kc == 0),
                                             stop=(kc == 1))
                        et = work.tile([128, NBLK], f32r, tag="wk",
                                       name=f"e{cb}_{nb}")
                        nc.vector.tensor_copy(et[:], ps[:])
                        ep.append(et)
                    # A = rh^T e (rows 0:64), B = rw^T e (rows 64:128) -> ab bf16
                    psa = psAB.tile([64, NBLK], f32, tag="ps", name=f"psa{nb}")
                    for kc in range(2):
                        nc.tensor.matmul(psa[:], rh_t[:, kc, :], ep[kc][:],
                                         start=(kc == 0), stop=(kc == 1))
                    nc.scalar.activation(ab_t[0:64, ns], psa[:], AF.Identity)
                    psb = psAB.tile([64, NBLK], f32, tag="ps", name=f"psb{nb}")
                    for kc in range(2):
                        nc.tensor.matmul(psb[:], rw_t[:, kc, :], ep[kc][:],
                                         start=(kc == 0), stop=(kc == 1))
                    nc.scalar.activation(ab_t[64:128, ns], psb[:], AF.Identity)
                    # vT tiles: vt[m, c] for the 4 m-blocks in this n-block
                    for sb in range(4):
                        m = nb * 4 + sb
                        msl = bass.ts(nb * 4 + sb, 128)  # columns of x1
                        ps = psAB.tile([128, C], f32, tag="ps", name=f"vps{m}")
                        nc.tensor.matmul(ps[:], x1_t[0][:, msl], vw_t[:, 0, :],
                                         start=True, stop=False)
                        nc.tensor.matmul(ps[:], x1_t[1][:, msl], vw_t[:, 1, :],
                                         start=False, stop=False)
                        nc.tensor.matmul(ps[:], ones1_t[:], vb_t[:],
                                         start=False, stop=True)
                        nc.vector.tensor_copy(vt_t[:, bass.ts(m, C)], ps[:])

            # =========== Phase C: attention (per n-pair) ======================
            out_t = [bigbuf.tile([128, N], f32r, tag="big", name=f"out_{c}")
                     for c in range(2)]
            with (
                tc.tile_pool(name="ps_s", bufs=2, space="PSUM") as ps_s,
                tc.tile_pool(name="ps_o", bufs=4, space="PSUM") as ps_o,
                tc.tile_pool(name="ps_n", bufs=2, space="PSUM") as ps_n,
            ):
                for pr in range(NNB // 2):
                    nbs0, nbs1 = 2 * pr, 2 * pr + 1
                    ops = [ps_o.tile([128, NBLK], f32, tag="oacc",
                                     name=f"oacc{pr}_{j}") for j in range(4)]
                    sps = [ps_n.tile([1, NBLK], f32, tag="nacc",
                                     name=f"nacc{pr}_{j}") for j in range(2)]
                    for mb in range(MB):
                        msl = bass.ts(mb, 128)
                        for j, nb in enumerate((nbs0, nbs1)):
                            ns = bass.ts(nb, NBLK)
                            st = ps_s.tile([128, NBLK], f32, tag="st",
                                           name=f"st{pr}_{mb}_{j}")
                            nc.tensor.matmul(st[:], ke_t[0][:, msl], qp_t[0][:, ns],
                                             start=True, stop=False)
                            nc.tensor.matmul(st[:], ke_t[1][:, msl], qp_t[1][:, ns],
                                             start=False, stop=False)
                            nc.tensor.matmul(st[:], ab_t[:, msl], oh_t[:, ns],
                                             start=False, stop=True)
                            es = work.tile([128, NBLK], f32r, tag="wk",
                                           name=f"es{pr}_{mb}_{j}")
                            nc.scalar.activation(es[:], st[:], AF.Exp)
                            last = (mb == MB - 1)
                            nc.tensor.matmul(ops[2 * j][:],
                                             vt_t[:, mb * C:mb * C + 128], es[:],
                                             start=(mb == 0), stop=last)
                            nc.tensor.matmul(ops[2 * j + 1][:],
                                             vt_t[:, mb * C + 128:mb * C + 256], es[:],
                                             start=(mb == 0), stop=last)
                            nc.tensor.matmul(sps[j][:], ones_t[:], es[:],
                                             start=(mb == 0), stop=last)
                    # normalize: out_norm = out_un / sums  (per n-block)
                    for j, nb in enumerate((nbs0, nbs1)):
                        ns = bass.ts(nb, NBLK)
                        rc = small.tile([1, NBLK], f32, tag="rc", name=f"rc{pr}_{j}")
                        nc.vector.reciprocal(rc[:], sps[j][:])
                        rcr = small.tile([1, NBLK], f32r, tag="rcr",
                                         name=f"rcr{pr}_{j}")
                        nc.vector.tensor_copy(rcr[:], rc[:])
                        bc = ps_s.tile([128, NBLK], f32, tag="st",
                                       name=f"bc{pr}_{j}")
                        nc.tensor.matmul(bc[:], ones1_t[:], rcr[:],
                                         start=True, stop=True)
                        bcs = work.tile([128, NBLK], f32, tag="wk",
                                        name=f"bcs{pr}_{j}")
                        nc.scalar.activation(bcs[:], bc[:], AF.Identity)
                        for cb in range(2):
                            nc.vector.tensor_mul(out_t[cb][:, ns],
                                                 ops[2 * j + cb][:], bcs[:])

                # =========== Phase D: y = silu(cv2' @ out + b2') + x ==========
                for nb in range(NNB):
                    ns = bass.ts(nb, NBLK)
                    for ob in range(4):
                        ps = ps_s.tile([128, NBLK], f32, tag="st",
                                       name=f"yps{nb}_{ob}")
                        for kc in range(2):
                            nc.tensor.matmul(ps[:], cv2_t[:, kc, bass.ts(ob, 128)],
                                             out_t[kc][:, ns], start=(kc == 0),
                                             stop=(kc == 1))
                        ysb = work.tile([128, NBLK], f32, tag="wk",
                                        name=f"ysb{nb}_{ob}")
                        nc.scalar.activation(ysb[:], ps[:], AF.Silu,
                                             bias=b2_t[:, ob:ob + 1])
                        x2 = xp_pool.tile([128, NBLK], f32, tag=f"x{ob}",
                                          name=f"x2_{nb}_{ob}")
                        nc.sync.dma_start(x2[:], x_d[bass.ts(ob, 128), ns])
                        res = work.tile([128, NBLK], f32, tag="wk",
                                        name=f"res{nb}_{ob}")
                        nc.vector.tensor_add(res[:], ysb[:], x2[:])
                        nc.sync.dma_start(y_d[bass.ts(ob, 128), ns], res[:])

    nc.compile()
    return nc


def prep_inputs(inputs):
    """Host-side folding of BN + weight layouts. Returns the shared in_map."""
    i = {k: np.asarray(v, dtype=np.float32) if np.asarray(v).dtype == np.float32
         else np.asarray(v) for k, v in inputs.items()}
    s1 = i["bn1_g"] / np.sqrt(i["bn1_v"] + EPS)
    cv1w = i["cv1_w"] * s1[:, None]                       # [C, DIMS]
    b1 = i["bn1_b"] - i["bn1_m"] * s1                     # [C]
    s2 = i["bn2_g"] / np.sqrt(i["bn2_v"] + EPS)
    cv2w = i["cv2_w"] * s2[:, None]                       # [DIMS, C]
    b2 = i["bn2_b"] - i["bn2_m"] * s2                     # [DIMS]

    n_idx = np.arange(N)
    onehot = np.zeros((128, N), np.float32)
    onehot[n_idx // SIZE, n_idx] = 1.0                    # H rows 0:64
    onehot[64 + n_idx % SIZE, n_idx] = 1.0                # W rows 64:128

    return {
        "cv1_lhsT": np.ascontiguousarray(cv1w.T),         # [DIMS, C]
        "b1": np.ascontiguousarray(b1.reshape(2, 128).T),
        "q_lhsT": np.ascontiguousarray(i["q_w"].T),
        "q_bias": np.ascontiguousarray(i["q_b"].reshape(2, 128).T),
        "k_lhsT": np.ascontiguousarray(i["k_w"].T),
        "e_lhsT": np.ascontiguousarray(i["e_w"].T),
        "v_rhs": np.ascontiguousarray(i["v_w"].T),        # [C, C]: v_rhs[ci,c]
        "v_bias_row": np.ascontiguousarray(i["v_b"].reshape(1, C)),
        "rh": np.ascontiguousarray(i["rel_h"].reshape(C, SIZE)),
        "rw": np.ascontiguousarray(i["rel_w"].reshape(C, SIZE)),
        "onehot": onehot.astype(ml_dtypes.bfloat16),
        "cv2_lhsT": np.ascontiguousarray(cv2w.T),         # [C, DIMS]
        "b2": np.ascontiguousarray(b2.reshape(4, 128).T),
        "ones_col": np.ones((128, 1), np.float32),
        "ones_row": np.ones((1, 128), np.float32),
    }


_NC = None


def run(inputs, trace=False):
    global _NC
    if _NC is None:
        _NC = build_nc()
    shared = prep_inputs(inputs)
    x = np.asarray(inputs["x"], dtype=np.float32)  # [B, DIMS, SIZE, SIZE]
    in_maps = []
    for b in range(B):
        m = dict(shared)
        m["x"] = np.ascontiguousarray(x[b].reshape(DIMS, N))
        in_maps.append(m)
    res = run_bass_kernel_spmd(_NC, in_maps, list(range(B)), trace=trace)
    out = np.stack([res.results[b]["y"].reshape(DIMS, SIZE, SIZE)
                    for b in range(B)], axis=0)
    return out.astype(np.float32), res.exec_time_ns


def kernel(**inputs) -> np.ndarray:
    out, _ = run(inputs, trace=False)
    return out


# revision 5
# speedup vs baseline: 1.0695x; 1.0695x over previous
"""Trainium2 Bass kernel for nn_ExtraPositionPromptSABottleneck.

Data-parallel over batch B=8 across 8 NeuronCores; each core computes one
batch element's full bottleneck block:

  x1 = silu(bn1(cv1 @ x))            [C=256, N=4096]
  q/k/e = proj(x1); v^T materialized directly via matmul
  S^T[m,n] = k^T q + (rel-pos term)  computed in transposed layout so that
             softmax-exp needs no max subtraction (scores are |s|<40) and
             the output matmul out = v @ attn^T needs no 4096^2 transpose.
  rel-pos:   pos^T e = H^T A + W^T B with A = rh^T e, B = rw^T e ([64,4096])
             and H/W constant one-hots -> packed as a third 128-row K-chunk
             [A;B] x [H;W] of the S matmul.
  softmax:   k_b/e_b biases drop out (softmax row-shift invariance);
             row sums via ones-vector matmul; normalization deferred to the
             PSUM->SBUF evacuation of the output accumulator.
  y = silu(bn2(cv2 @ out_norm)) + x

All matmuls run in float32r (single-pass fp32 at bf16 rate); the one-hot
chunk runs in bf16 (exact 0/1 values).
"""

import os
import numpy as np
import ml_dtypes

import concourse.bass as bass
import concourse.tile as tile
from concourse import bacc, mybir
from concourse.bass_utils import run_bass_kernel_spmd

f32 = mybir.dt.float32
f32r = mybir.dt.float32r
bf16 = mybir.dt.bfloat16
AF = mybir.ActivationFunctionType

B, DIMS, SIZE = 8, 512, 64
C = DIMS // 2              # 256
N = SIZE * SIZE            # 4096
NBLK = 512                 # column block (one PSUM bank of fp32)
NNB = N // NBLK            # 8 n blocks
MB = N // 128              # 32 m blocks
EPS = 1e-5


def build_nc():
    nc = bacc.Bacc("TRN2", target_bir_lowering=False, debug=False)

    x_d = nc.dram_tensor("x", [DIMS, N], f32, kind="ExternalInput")
    cv1_d = nc.dram_tensor("cv1_lhsT", [DIMS, C], f32, kind="ExternalInput")
    b1_d = nc.dram_tensor("b1", [128, 2], f32, kind="ExternalInput")
    qw_d = nc.dram_tensor("q_lhsT", [C, C], f32, kind="ExternalInput")
    qb_d = nc.dram_tensor("q_bias", [128, 2], f32, kind="ExternalInput")
    kw_d = nc.dram_tensor("k_lhsT", [C, C], f32, kind="ExternalInput")
    ew_d = nc.dram_tensor("e_lhsT", [C, C], f32, kind="ExternalInput")
    vw_d = nc.dram_tensor("v_rhs", [C, C], f32, kind="ExternalInput")
    vb_d = nc.dram_tensor("v_bias_row", [1, C], f32, kind="ExternalInput")
    rh_d = nc.dram_tensor("rh", [C, SIZE], f32, kind="ExternalInput")
    rw_d = nc.dram_tensor("rw", [C, SIZE], f32, kind="ExternalInput")
    oh_d = nc.dram_tensor("onehot", [128, N], bf16, kind="ExternalInput")
    cv2_d = nc.dram_tensor("cv2_lhsT", [C, DIMS], f32, kind="ExternalInput")
    b2_d = nc.dram_tensor("b2", [128, 4], f32, kind="ExternalInput")
    ones_d = nc.dram_tensor("ones_col", [128, 1], f32, kind="ExternalInput")
    ones1_d = nc.dram_tensor("ones_row", [1, 128], f32, kind="ExternalInput")
    y_d = nc.dram_tensor("y", [DIMS, N], f32, kind="ExternalOutput")

    with tile.TileContext(nc) as tc:
        with (
            tc.tile_pool(name="wp", bufs=1) as wp,
            tc.tile_pool(name="qp", bufs=1) as qp_pool,
            tc.tile_pool(name="kep", bufs=1) as ke_pool,
            tc.tile_pool(name="vtp", bufs=1) as vt_pool,
            tc.tile_pool(name="bigbuf", bufs=2) as bigbuf,
            tc.tile_pool(name="xpanels", bufs=2) as xp_pool,
            tc.tile_pool(name="work", bufs=6) as work,
            tc.tile_pool(name="small", bufs=2) as small,
        ):
            # ---- weights / constants ----
            cv1_t = wp.tile([128, 4, C], f32r, tag="cv1_t")
            nc.gpsimd.dma_start(cv1_t[:], cv1_d.rearrange("(k p) m -> p k m", p=128))
            qw_t = wp.tile([128, 2, C], f32r, tag="qw_t")
            nc.gpsimd.dma_start(qw_t[:], qw_d.rearrange("(k p) m -> p k m", p=128))
            kw_t = wp.tile([128, 2, C], f32r, tag="kw_t")
            nc.gpsimd.dma_start(kw_t[:], kw_d.rearrange("(k p) m -> p k m", p=128))
            ew_t = wp.tile([128, 2, C], f32r, tag="ew_t")
            nc.gpsimd.dma_start(ew_t[:], ew_d.rearrange("(k p) m -> p k m", p=128))
            vw_t = wp.tile([128, 2, C], f32r, tag="vw_t")
            nc.gpsimd.dma_start(vw_t[:], vw_d.rearrange("(k p) m -> p k m", p=128))
            vb_t = wp.tile([1, C], f32r, tag="vb_t")
            nc.gpsimd.dma_start(vb_t[:], vb_d[:])
            rh_t = wp.tile([128, 2, SIZE], f32r, tag="rh_t")
            nc.gpsimd.dma_start(rh_t[:], rh_d.rearrange("(k p) m -> p k m", p=128))
            rw_t = wp.tile([128, 2, SIZE], f32r, tag="rw_t")
            nc.gpsimd.dma_start(rw_t[:], rw_d.rearrange("(k p) m -> p k m", p=128))
            cv2_t = wp.tile([128, 2, DIMS], f32r, tag="cv2_t")
            nc.gpsimd.dma_start(cv2_t[:], cv2_d.rearrange("(k p) m -> p k m", p=128))
            ones_t = wp.tile([128, 1], f32r, tag="ones_t")
            nc.gpsimd.dma_start(ones_t[:], ones_d[:])
            ones1_t = wp.tile([1, 128], f32r, tag="ones1_t")
            nc.gpsimd.dma_start(ones1_t[:], ones1_d[:])
            b1_t = wp.tile([128, 2], f32, tag="b1_t")
            nc.sync.dma_start(b1_t[:], b1_d[:])
            qb_t = wp.tile([128, 2], f32, tag="qb_t")
            nc.sync.dma_start(qb_t[:], qb_d[:])
            b2_t = wp.tile([128, 4], f32, tag="b2_t")
            nc.sync.dma_start(b2_t[:], b2_d[:])
            oh_t = wp.tile([128, N], bf16, tag="oh_t")
            nc.sync.dma_start(oh_t[:], oh_d[:])

            # ---- persistent big tensors ----
            qp_t = [qp_pool.tile([128, N], f32r, tag=f"qp{c}", name=f"qp{c}")
                    for c in range(2)]
            ke_t = [ke_pool.tile([128, N], f32r, tag=f"ke{c}", name=f"ke{c}")
                    for c in range(2)]
            ab_t = ke_pool.tile([128, N], bf16, tag="ab")
            vt_t = vt_pool.tile([128, MB * C], f32r, tag="vt")
            x1_t = [bigbuf.tile([128, N], f32r, tag="big", name=f"x1_{c}")
                    for c in range(2)]

            # =========== Phase A+B: x -> x1 -> q,k,e,AB,vT (per n-block) ======
            with tc.tile_pool(name="psAB", bufs=6, space="PSUM") as psAB:
                for nb in range(NNB):
                    ns = bass.ts(nb, NBLK)
                    xp = []
                    for kc in range(4):
                        xt = xp_pool.tile([128, NBLK], f32r, tag=f"x{kc}",
                                          name=f"xp{kc}_{nb}")
                        nc.gpsimd.dma_start(xt[:], x_d[bass.ts(kc, 128), ns])
                        xp.append(xt)
                    # x1 = silu(cv1' @ x + b1')
                    for cb in range(2):
                        ps = psAB.tile([128, NBLK], f32, tag="ps", name=f"x1ps{nb}_{cb}")
                        for kc in range(4):
                            nc.tensor.matmul(ps[:], cv1_t[:, kc, bass.ts(cb, 128)],
                                             xp[kc][:], start=(kc == 0), stop=(kc == 3))
                        nc.scalar.activation(x1_t[cb][:, ns], ps[:], AF.Silu,
                                             bias=b1_t[:, cb:cb + 1])
                    # q (bias q_b), k (no bias)
                    for cb in range(2):
                        ps = psAB.tile([128, NBLK], f32, tag="ps", name=f"qps{nb}_{cb}")
                        for kc in range(2):
                            nc.tensor.matmul(ps[:], qw_t[:, kc, bass.ts(cb, 128)],
                                             x1_t[kc][:, ns], start=(kc == 0),
                                             stop=(kc == 1))
                        nc.scalar.activation(qp_t[cb][:, ns], ps[:], AF.Identity,
                                             bias=qb_t[:, cb:cb + 1])
                    for cb in range(2):
                        ps = psAB.tile([128, NBLK], f32, tag="ps", name=f"kps{nb}_{cb}")
                        for kc in range(2):
                            nc.tensor.matmul(ps[:], kw_t[:, kc, bass.ts(cb, 128)],
                                             x1_t[kc][:, ns], start=(kc == 0),
                                             stop=(kc == 1))
                        nc.vector.tensor_copy(ke_t[cb][:, ns], ps[:])
                    # e panels (consumed immediately by A/B matmuls)
                    ep = []
                    for cb in range(2):
                        ps = psAB.tile([128, NBLK], f32, tag="ps", name=f"eps{nb}_{cb}")
                        for kc in range(2):
                            nc.tensor.matmul(ps[:], ew_t[:, kc, bass.ts(cb, 128)],
                                             x1_t[kc][:, ns], start=(kc == 0),
                                             stop=(kc == 1))
                        et = work.tile([128, NBLK], f32r, tag="wk",
                                       name=f"e{cb}_{nb}")
                        nc.vector.tensor_copy(et[:], ps[:])
                        ep.append(et)
                    # A = rh^T e (rows 0:64), B = rw^T e (rows 64:128) -> ab bf16
                    psa = psAB.tile([64, NBLK], f32, tag="ps", name=f"psa{nb}")
                    for kc in range(2):
                        nc.tensor.matmul(psa[:], rh_t[:, kc, :], ep[kc][:],
                                         start=(kc == 0), stop=(kc == 1))
                    nc.scalar.activation(ab_t[0:64, ns], psa[:], AF.Identity)
                    psb = psAB.tile([64, NBLK], f32, tag="ps", name=f"psb{nb}")
                    for kc in range(2):
                        nc.tensor.matmul(psb[:], rw_t[:, kc, :], ep[kc][:],
                                         start=(kc == 0), stop=(kc == 1))
                    nc.scalar.activation(ab_t[64:128, ns], psb[:], AF.Identity)
                    # vT tiles: vt[m, c] for the 4 m-blocks in this n-block
                    for sb in range(4):
                        m = nb * 4 + sb
                        msl = bass.ts(nb * 4 + sb, 128)  # columns of x1
                        ps = psAB.tile([128, C], f32, tag="ps", name=f"vps{m}")
                        nc.tensor.matmul(ps[:], x1_t[0][:, msl], vw_t[:, 0, :],
                                         start=True, stop=False)
                        nc.tensor.matmul(ps[:], x1_t[1][:, msl], vw_t[:, 1, :],
                                         start=False, stop=False)
                        nc.tensor.matmul(ps[:], ones1_t[:], vb_t[:],
                                         start=False, stop=True)
                        nc.vector.tensor_copy(vt_t[:, bass.ts(m, C)], ps[:])

            # =========== Phase C: attention (per n-pair) ======================
            out_t = [bigbuf.tile([128, N], f32r, tag="big", name=f"out_{c}")
                     for c in range(2)]
            with (
                tc.tile_pool(name="ps_s", bufs=2, space="PSUM") as ps_s,
                tc.tile_pool(name="ps_o", bufs=4, space="PSUM") as ps_o,
                tc.tile_pool(name="ps_n", bufs=2, space="PSUM") as ps_n,
            ):
                for pr in range(NNB // 2):
                    nbs0, nbs1 = 2 * pr, 2 * pr + 1
                    ops = [ps_o.tile([128, NBLK], f32, tag="oacc",
                                     name=f"oacc{pr}_{j}") for j in range(4)]
                    sps = [ps_n.tile([1, NBLK], f32, tag="nacc",
                                     name=f"nacc{pr}_{j}") for j in range(2)]
                    # software-pipelined: out-matmuls run one m-block behind
                    # the exp that produces their rhs, so PE never waits on ACT.
                    es_prev = [None, None]
                    def emit_out(mb, es_pair):
                        last = (mb == MB - 1)
                        for j in range(2):
                            nc.tensor.matmul(ops[2 * j][:],
                                             vt_t[:, mb * C:mb * C + 128],
                                             es_pair[j][:],
                                             start=(mb == 0), stop=last)
                            nc.tensor.matmul(ops[2 * j + 1][:],
                                             vt_t[:, mb * C + 128:mb * C + 256],
                                             es_pair[j][:],
                                             start=(mb == 0), stop=last)
                            nc.tensor.matmul(sps[j][:], ones_t[:], es_pair[j][:],
                                             start=(mb == 0), stop=last)
                    for mb in range(MB):
                        msl = bass.ts(mb, 128)
                        es_cur = []
                        for j, nb in enumerate((nbs0, nbs1)):
                            ns = bass.ts(nb, NBLK)
                            st = ps_s.tile([128, NBLK], f32, tag="st",
                                           name=f"st{pr}_{mb}_{j}")
                            nc.tensor.matmul(st[:], ke_t[0][:, msl], qp_t[0][:, ns],
                                             start=True, stop=False)
                            nc.tensor.matmul(st[:], ke_t[1][:, msl], qp_t[1][:, ns],
                                             start=False, stop=False)
                            nc.tensor.matmul(st[:], ab_t[:, msl], oh_t[:, ns],
                                             start=False, stop=True)
                            es = work.tile([128, NBLK], f32r, tag="wk",
                                           name=f"es{pr}_{mb}_{j}")
                            nc.scalar.activation(es[:], st[:], AF.Exp)
                            es_cur.append(es)
                        if mb > 0:
                            emit_out(mb - 1, es_prev)
                        es_prev = es_cur
                    emit_out(MB - 1, es_prev)
                    # Evacuate accumulators via ACT immediately (frees PSUM for
                    # the next pair); the reciprocal/normalize chain then runs
                    # on DVE fully overlapped with the next pair's attention.
                    for j, nb in enumerate((nbs0, nbs1)):
                        ns = bass.ts(nb, NBLK)
                        for cb in range(2):
                            nc.scalar.activation(out_t[cb][:, ns],
                                                 ops[2 * j + cb][:], AF.Identity)
                        ssb = small.tile([1, NBLK], f32, tag="ssb",
                                         name=f"ssb{pr}_{j}")
                        nc.scalar.activation(ssb[:], sps[j][:], AF.Identity)
                        rc = small.tile([1, NBLK], f32, tag="rc", name=f"rc{pr}_{j}")
                        nc.vector.reciprocal(rc[:], ssb[:])
                        rcr = small.tile([1, NBLK], f32r, tag="rcr",
                                         name=f"rcr{pr}_{j}")
                        nc.vector.tensor_copy(rcr[:], rc[:])
                        bc = ps_s.tile([128, NBLK], f32, tag="st",
                                       name=f"bc{pr}_{j}")
                        nc.tensor.matmul(bc[:], ones1_t[:], rcr[:],
                                         start=True, stop=True)
                        bcs = work.tile([128, NBLK], f32, tag="wk",
                                        name=f"bcs{pr}_{j}")
                        nc.scalar.activation(bcs[:], bc[:], AF.Identity)
                        for cb in range(2):
                            nc.vector.tensor_mul(out_t[cb][:, ns],
                                                 out_t[cb][:, ns], bcs[:])

                # =========== Phase D: y = silu(cv2' @ out + b2') + x ==========
                for nb in range(NNB):
                    ns = bass.ts(nb, NBLK)
                    for ob in range(4):
                        ps = ps_s.tile([128, NBLK], f32, tag="st",
                                       name=f"yps{nb}_{ob}")
                        for kc in range(2):
                            nc.tensor.matmul(ps[:], cv2_t[:, kc, bass.ts(ob, 128)],
                                             out_t[kc][:, ns], start=(kc == 0),
                                             stop=(kc == 1))
                        ysb = work.tile([128, NBLK], f32, tag="wk",
                                        name=f"ysb{nb}_{ob}")
                        nc.scalar.activation(ysb[:], ps[:], AF.Silu,
                                             bias=b2_t[:, ob:ob + 1])
                        x2 = xp_pool.tile([128, NBLK], f32, tag=f"x{ob}",
                                          name=f"x2_{nb}_{ob}")
                        nc.sync.dma_start(x2[:], x_d[bass.ts(ob, 128), ns])
                        res = work.tile([128, NBLK], f32, tag="wk",
                                        name=f"res{nb}_{ob}")
                        nc.vector.tensor_add(res[:], ysb[:], x2[:])
                        nc.sync.dma_start(y_d[bass.ts(ob, 128), ns], res[:])

    nc.compile()
    return nc


def prep_inputs(inputs):
    """Host-side folding of BN + weight layouts. Returns the shared in_map."""
    i = {k: np.asarray(v, dtype=np.float32) if np.asarray(v).dtype == np.float32
         else np.asarray(v) for k, v in inputs.items()}
    s1 = i["bn1_g"] / np.sqrt(i["bn1_v"] + EPS)
    cv1w = i["cv1_w"] * s1[:, None]                       # [C, DIMS]
    b1 = i["bn1_b"] - i["bn1_m"] * s1                     # [C]
    s2 = i["bn2_g"] / np.sqrt(i["bn2_v"] + EPS)
    cv2w = i["cv2_w"] * s2[:, None]                       # [DIMS, C]
    b2 = i["bn2_b"] - i["bn2_m"] * s2                     # [DIMS]

    n_idx = np.arange(N)
    onehot = np.zeros((128, N), np.float32)
    onehot[n_idx // SIZE, n_idx] = 1.0                    # H rows 0:64
    onehot[64 + n_idx % SIZE, n_idx] = 1.0                # W rows 64:128

    return {
        "cv1_lhsT": np.ascontiguousarray(cv1w.T),         # [DIMS, C]
        "b1": np.ascontiguousarray(b1.reshape(2, 128).T),
        "q_lhsT": np.ascontiguousarray(i["q_w"].T),
        "q_bias": np.ascontiguousarray(i["q_b"].reshape(2, 128).T),
        "k_lhsT": np.ascontiguousarray(i["k_w"].T),
        "e_lhsT": np.ascontiguousarray(i["e_w"].T),
        "v_rhs": np.ascontiguousarray(i["v_w"].T),        # [C, C]: v_rhs[ci,c]
        "v_bias_row": np.ascontiguousarray(i["v_b"].reshape(1, C)),
        "rh": np.ascontiguousarray(i["rel_h"].reshape(C, SIZE)),
        "rw": np.ascontiguousarray(i["rel_w"].reshape(C, SIZE)),
        "onehot": onehot.astype(ml_dtypes.bfloat16),
        "cv2_lhsT": np.ascontiguousarray(cv2w.T),         # [C, DIMS]
        "b2": np.ascontiguousarray(b2.reshape(4, 128).T),
        "ones_col": np.ones((128, 1), np.float32),
        "ones_row": np.ones((1, 128), np.float32),
    }


_NC = None


def run(inputs, trace=False):
    global _NC
    if _NC is None:
        _NC = build_nc()
    shared = prep_inputs(inputs)
    x = np.asarray(inputs["x"], dtype=np.float32)  # [B, DIMS, SIZE, SIZE]
    in_maps = []
    for b in range(B):
        m = dict(shared)
        m["x"] = np.ascontiguousarray(x[b].reshape(DIMS, N))
        in_maps.append(m)
    res = run_bass_kernel_spmd(_NC, in_maps, list(range(B)), trace=trace)
    out = np.stack([res.results[b]["y"].reshape(DIMS, SIZE, SIZE)
                    for b in range(B)], axis=0)
    return out.astype(np.float32), res.exec_time_ns


def kernel(**inputs) -> np.ndarray:
    out, _ = run(inputs, trace=False)
    return out


# revision 6
# speedup vs baseline: 1.1255x; 1.0524x over previous
"""Trainium2 Bass kernel for nn_ExtraPositionPromptSABottleneck.

Data-parallel over batch B=8 across 8 NeuronCores; each core computes one
batch element's full bottleneck block:

  x1 = silu(bn1(cv1 @ x))            [C=256, N=4096]
  q/k/e = proj(x1); v^T materialized directly via matmul
  S^T[m,n] = k^T q + (rel-pos term)  computed in transposed layout so that
             softmax-exp needs no max subtraction (scores are |s|<40) and
             the output matmul out = v @ attn^T needs no 4096^2 transpose.
  rel-pos:   pos^T e = H^T A + W^T B with A = rh^T e, B = rw^T e ([64,4096])
             and H/W constant one-hots -> packed as a third 128-row K-chunk
             [A;B] x [H;W] of the S matmul.
  softmax:   k_b/e_b biases drop out (softmax row-shift invariance);
             row sums via ones-vector matmul; normalization deferred to the
             PSUM->SBUF evacuation of the output accumulator.
  y = silu(bn2(cv2 @ out_norm)) + x

All matmuls run in float32r (single-pass fp32 at bf16 rate); the one-hot
chunk runs in bf16 (exact 0/1 values).
"""

import os
import numpy as np
import ml_dtypes

import concourse.bass as bass
import concourse.tile as tile
from concourse import bacc, mybir
from concourse.bass_utils import run_bass_kernel_spmd

f32 = mybir.dt.float32
f32r = mybir.dt.float32r
bf16 = mybir.dt.bfloat16
AF = mybir.ActivationFunctionType

B, DIMS, SIZE = 8, 512, 64
C = DIMS // 2              # 256
N = SIZE * SIZE            # 4096
NBLK = 512                 # column block (one PSUM bank of fp32)
NNB = N // NBLK            # 8 n blocks
MB = N // 128              # 32 m blocks
EPS = 1e-5


def build_nc():
    nc = bacc.Bacc("TRN2", target_bir_lowering=False, debug=False)

    x_d = nc.dram_tensor("x", [DIMS, N], f32, kind="ExternalInput")
    cv1_d = nc.dram_tensor("cv1_lhsT", [DIMS, C], f32, kind="ExternalInput")
    b1_d = nc.dram_tensor("b1", [128, 2], f32, kind="ExternalInput")
    qw_d = nc.dram_tensor("q_lhsT", [C, C], f32, kind="ExternalInput")
    qb_d = nc.dram_tensor("q_bias", [128, 2], f32, kind="ExternalInput")
    kw_d = nc.dram_tensor("k_lhsT", [C, C], f32, kind="ExternalInput")
    ew_d = nc.dram_tensor("e_lhsT", [C, C], f32, kind="ExternalInput")
    vw_d = nc.dram_tensor("v_rhs", [C, C], f32, kind="ExternalInput")
    vb_d = nc.dram_tensor("v_bias_row", [1, C], f32, kind="ExternalInput")
    rh_d = nc.dram_tensor("rh", [C, SIZE], f32, kind="ExternalInput")
    rw_d = nc.dram_tensor("rw", [C, SIZE], f32, kind="ExternalInput")
    oh_d = nc.dram_tensor("onehot", [128, N], bf16, kind="ExternalInput")
    cv2_d = nc.dram_tensor("cv2_lhsT", [C, DIMS], f32, kind="ExternalInput")
    b2_d = nc.dram_tensor("b2", [128, 4], f32, kind="ExternalInput")
    ones_d = nc.dram_tensor("ones_col", [128, 1], f32, kind="ExternalInput")
    ones1_d = nc.dram_tensor("ones_row", [1, 128], f32, kind="ExternalInput")
    y_d = nc.dram_tensor("y", [DIMS, N], f32, kind="ExternalOutput")

    with tile.TileContext(nc) as tc:
        with (
            tc.tile_pool(name="wp", bufs=1) as wp,
            tc.tile_pool(name="qp", bufs=1) as qp_pool,
            tc.tile_pool(name="kep", bufs=1) as ke_pool,
            tc.tile_pool(name="vtp", bufs=1) as vt_pool,
            tc.tile_pool(name="bigbuf", bufs=2) as bigbuf,
            tc.tile_pool(name="xpanels", bufs=2) as xp_pool,
            tc.tile_pool(name="work", bufs=6) as work,
            tc.tile_pool(name="small", bufs=2) as small,
            tc.tile_pool(name="small2", bufs=4) as small2,
        ):
            # ---- weights / constants ----
            cv1_t = wp.tile([128, 4, C], f32r, tag="cv1_t")
            nc.gpsimd.dma_start(cv1_t[:], cv1_d.rearrange("(k p) m -> p k m", p=128))
            qw_t = wp.tile([128, 2, C], f32r, tag="qw_t")
            nc.gpsimd.dma_start(qw_t[:], qw_d.rearrange("(k p) m -> p k m", p=128))
            kw_t = wp.tile([128, 2, C], f32r, tag="kw_t")
            nc.gpsimd.dma_start(kw_t[:], kw_d.rearrange("(k p) m -> p k m", p=128))
            ew_t = wp.tile([128, 2, C], f32r, tag="ew_t")
            nc.gpsimd.dma_start(ew_t[:], ew_d.rearrange("(k p) m -> p k m", p=128))
            vw_t = wp.tile([128, 2, C], f32r, tag="vw_t")
            nc.gpsimd.dma_start(vw_t[:], vw_d.rearrange("(k p) m -> p k m", p=128))
            vb_t = wp.tile([1, C], f32r, tag="vb_t")
            nc.gpsimd.dma_start(vb_t[:], vb_d[:])
            rh_t = wp.tile([128, 2, SIZE], f32r, tag="rh_t")
            nc.gpsimd.dma_start(rh_t[:], rh_d.rearrange("(k p) m -> p k m", p=128))
            rw_t = wp.tile([128, 2, SIZE], f32r, tag="rw_t")
            nc.gpsimd.dma_start(rw_t[:], rw_d.rearrange("(k p) m -> p k m", p=128))
            cv2_t = wp.tile([128, 2, DIMS], f32r, tag="cv2_t")
            nc.gpsimd.dma_start(cv2_t[:], cv2_d.rearrange("(k p) m -> p k m", p=128))
            ones_t = wp.tile([128, 1], f32r, tag="ones_t")
            nc.gpsimd.dma_start(ones_t[:], ones_d[:])
            ones1_t = wp.tile([1, 128], f32r, tag="ones1_t")
            nc.gpsimd.dma_start(ones1_t[:], ones1_d[:])
            b1_t = wp.tile([128, 2], f32, tag="b1_t")
            nc.sync.dma_start(b1_t[:], b1_d[:])
            qb_t = wp.tile([128, 2], f32, tag="qb_t")
            nc.sync.dma_start(qb_t[:], qb_d[:])
            b2_t = wp.tile([128, 4], f32, tag="b2_t")
            nc.sync.dma_start(b2_t[:], b2_d[:])
            oh_t = wp.tile([128, N], bf16, tag="oh_t")
            nc.sync.dma_start(oh_t[:], oh_d[:])

            # ---- persistent big tensors ----
            qp_t = [qp_pool.tile([128, N], f32r, tag=f"qp{c}", name=f"qp{c}")
                    for c in range(2)]
            ke_t = [ke_pool.tile([128, N], f32r, tag=f"ke{c}", name=f"ke{c}")
                    for c in range(2)]
            ab_t = ke_pool.tile([128, N], bf16, tag="ab")
            vt_t = vt_pool.tile([128, MB * C], f32r, tag="vt")
            x1_t = [bigbuf.tile([128, N], f32r, tag="big", name=f"x1_{c}")
                    for c in range(2)]

            # =========== Phase A+B: x -> x1 -> q,k,e,AB,vT (per n-block) ======
            with tc.tile_pool(name="psAB", bufs=6, space="PSUM") as psAB:
                for nb in range(NNB):
                    ns = bass.ts(nb, NBLK)
                    xp = []
                    for kc in range(4):
                        xt = xp_pool.tile([128, NBLK], f32r, tag=f"x{kc}",
                                          name=f"xp{kc}_{nb}")
                        nc.gpsimd.dma_start(xt[:], x_d[bass.ts(kc, 128), ns])
                        xp.append(xt)
                    # x1 = silu(cv1' @ x + b1')
                    for cb in range(2):
                        ps = psAB.tile([128, NBLK], f32, tag="ps", name=f"x1ps{nb}_{cb}")
                        for kc in range(4):
                            nc.tensor.matmul(ps[:], cv1_t[:, kc, bass.ts(cb, 128)],
                                             xp[kc][:], start=(kc == 0), stop=(kc == 3))
                        nc.scalar.activation(x1_t[cb][:, ns], ps[:], AF.Silu,
                                             bias=b1_t[:, cb:cb + 1])
                    # q (bias q_b), k (no bias)
                    for cb in range(2):
                        ps = psAB.tile([128, NBLK], f32, tag="ps", name=f"qps{nb}_{cb}")
                        for kc in range(2):
                            nc.tensor.matmul(ps[:], qw_t[:, kc, bass.ts(cb, 128)],
                                             x1_t[kc][:, ns], start=(kc == 0),
                                             stop=(kc == 1))
                        nc.scalar.activation(qp_t[cb][:, ns], ps[:], AF.Identity,
                                             bias=qb_t[:, cb:cb + 1])
                    for cb in range(2):
                        ps = psAB.tile([128, NBLK], f32, tag="ps", name=f"kps{nb}_{cb}")
                        for kc in range(2):
                            nc.tensor.matmul(ps[:], kw_t[:, kc, bass.ts(cb, 128)],
                                             x1_t[kc][:, ns], start=(kc == 0),
                                             stop=(kc == 1))
                        nc.vector.tensor_copy(ke_t[cb][:, ns], ps[:])
                    # e panels (consumed immediately by A/B matmuls)
                    ep = []
                    for cb in range(2):
                        ps = psAB.tile([128, NBLK], f32, tag="ps", name=f"eps{nb}_{cb}")
                        for kc in range(2):
                            nc.tensor.matmul(ps[:], ew_t[:, kc, bass.ts(cb, 128)],
                                             x1_t[kc][:, ns], start=(kc == 0),
                                             stop=(kc == 1))
                        et = work.tile([128, NBLK], f32r, tag="wk",
                                       name=f"e{cb}_{nb}")
                        nc.vector.tensor_copy(et[:], ps[:])
                        ep.append(et)
                    # A = rh^T e (rows 0:64), B = rw^T e (rows 64:128) -> ab bf16
                    psa = psAB.tile([64, NBLK], f32, tag="ps", name=f"psa{nb}")
                    for kc in range(2):
                        nc.tensor.matmul(psa[:], rh_t[:, kc, :], ep[kc][:],
                                         start=(kc == 0), stop=(kc == 1))
                    nc.scalar.activation(ab_t[0:64, ns], psa[:], AF.Identity)
                    psb = psAB.tile([64, NBLK], f32, tag="ps", name=f"psb{nb}")
                    for kc in range(2):
                        nc.tensor.matmul(psb[:], rw_t[:, kc, :], ep[kc][:],
                                         start=(kc == 0), stop=(kc == 1))
                    nc.scalar.activation(ab_t[64:128, ns], psb[:], AF.Identity)
                    # vT tiles: vt[m, c] for the 4 m-blocks in this n-block
                    for sb in range(4):
                        m = nb * 4 + sb
                        msl = bass.ts(nb * 4 + sb, 128)  # columns of x1
                        ps = psAB.tile([128, C], f32, tag="ps", name=f"vps{m}")
                        nc.tensor.matmul(ps[:], x1_t[0][:, msl], vw_t[:, 0, :],
                                         start=True, stop=False)
                        nc.tensor.matmul(ps[:], x1_t[1][:, msl], vw_t[:, 1, :],
                                         start=False, stop=False)
                        nc.tensor.matmul(ps[:], ones1_t[:], vb_t[:],
                                         start=False, stop=True)
                        nc.vector.tensor_copy(vt_t[:, bass.ts(m, C)], ps[:])

            # =========== Phase C: attention (per n-pair) ======================
            out_t = [bigbuf.tile([128, N], f32r, tag="big", name=f"out_{c}")
                     for c in range(2)]
            with (
                tc.tile_pool(name="ps_s", bufs=2, space="PSUM") as ps_s,
                tc.tile_pool(name="ps_o", bufs=4, space="PSUM") as ps_o,
                tc.tile_pool(name="ps_n", bufs=2, space="PSUM") as ps_n,
            ):
                pending_norm = None
                for pr in range(NNB // 2):
                    nbs0, nbs1 = 2 * pr, 2 * pr + 1
                    ops = [ps_o.tile([128, NBLK], f32, tag="oacc",
                                     name=f"oacc{pr}_{j}") for j in range(4)]
                    sps = [ps_n.tile([1, NBLK], f32, tag="nacc",
                                     name=f"nacc{pr}_{j}") for j in range(2)]
                    # software-pipelined: out-matmuls run one m-block behind
                    # the exp that produces their rhs, so PE never waits on ACT.
                    es_prev = [None, None]
                    def emit_out(mb, es_pair):
                        last = (mb == MB - 1)
                        for j in range(2):
                            nc.tensor.matmul(ops[2 * j][:],
                                             vt_t[:, mb * C:mb * C + 128],
                                             es_pair[j][:],
                                             start=(mb == 0), stop=last)
                            nc.tensor.matmul(ops[2 * j + 1][:],
                                             vt_t[:, mb * C + 128:mb * C + 256],
                                             es_pair[j][:],
                                             start=(mb == 0), stop=last)
                            nc.tensor.matmul(sps[j][:], ones_t[:], es_pair[j][:],
                                             start=(mb == 0), stop=last)
                    for mb in range(MB):
                        msl = bass.ts(mb, 128)
                        es_cur = []
                        for j, nb in enumerate((nbs0, nbs1)):
                            ns = bass.ts(nb, NBLK)
                            st = ps_s.tile([128, NBLK], f32, tag="st",
                                           name=f"st{pr}_{mb}_{j}")
                            nc.tensor.matmul(st[:], ke_t[0][:, msl], qp_t[0][:, ns],
                                             start=True, stop=False)
                            nc.tensor.matmul(st[:], ke_t[1][:, msl], qp_t[1][:, ns],
                                             start=False, stop=False)
                            nc.tensor.matmul(st[:], ab_t[:, msl], oh_t[:, ns],
                                             start=False, stop=True)
                            es = work.tile([128, NBLK], f32r, tag="wk",
                                           name=f"es{pr}_{mb}_{j}")
                            nc.scalar.activation(es[:], st[:], AF.Exp)
                            es_cur.append(es)
                        if mb > 0:
                            emit_out(mb - 1, es_prev)
                        if mb == 4 and pending_norm is not None:
                            pending_norm()
                            pending_norm = None
                        es_prev = es_cur
                    emit_out(MB - 1, es_prev)
                    # Evacuate sums first (unblocks the reciprocal), then the
                    # accumulators (frees PSUM for the next pair). The
                    # PE-touching part of normalization (broadcast matmul) is
                    # deferred into the next pair's m-loop so it never
                    # head-of-line-blocks PE on the reciprocal chain.
                    rcrs = []
                    for j, nb in enumerate((nbs0, nbs1)):
                        ssb = small.tile([1, NBLK], f32, tag="ssb",
                                         name=f"ssb{pr}_{j}")
                        nc.scalar.activation(ssb[:], sps[j][:], AF.Identity)
                        rc = small.tile([1, NBLK], f32, tag="rc", name=f"rc{pr}_{j}")
                        nc.vector.reciprocal(rc[:], ssb[:])
                        rcr = small2.tile([1, NBLK], f32r, tag="rcr",
                                          name=f"rcr{pr}_{j}")
                        nc.vector.tensor_copy(rcr[:], rc[:])
                        rcrs.append(rcr)
                    for j, nb in enumerate((nbs0, nbs1)):
                        ns = bass.ts(nb, NBLK)
                        for cb in range(2):
                            nc.scalar.activation(out_t[cb][:, ns],
                                                 ops[2 * j + cb][:], AF.Identity)

                    def make_norm(pr, nbs, rcrs):
                        def norm():
                            for j, nb in enumerate(nbs):
                                ns = bass.ts(nb, NBLK)
                                bc = ps_s.tile([128, NBLK], f32, tag="st",
                                               name=f"bc{pr}_{j}")
                                nc.tensor.matmul(bc[:], ones1_t[:], rcrs[j][:],
                                                 start=True, stop=True)
                                bcs = work.tile([128, NBLK], f32, tag="wk",
                                                name=f"bcs{pr}_{j}")
                                nc.scalar.activation(bcs[:], bc[:], AF.Identity)
                                for cb in range(2):
                                    nc.vector.tensor_mul(out_t[cb][:, ns],
                                                         out_t[cb][:, ns], bcs[:])
                        return norm
                    pending_norm = make_norm(pr, (nbs0, nbs1), rcrs)

                if pending_norm is not None:
                    pending_norm()
                    pending_norm = None

                # =========== Phase D: y = silu(cv2' @ out + b2') + x ==========
                for nb in range(NNB):
                    ns = bass.ts(nb, NBLK)
                    for ob in range(4):
                        ps = ps_s.tile([128, NBLK], f32, tag="st",
                                       name=f"yps{nb}_{ob}")
                        for kc in range(2):
                            nc.tensor.matmul(ps[:], cv2_t[:, kc, bass.ts(ob, 128)],
                                             out_t[kc][:, ns], start=(kc == 0),
                                             stop=(kc == 1))
                        ysb = work.tile([128, NBLK], f32, tag="wk",
                                        name=f"ysb{nb}_{ob}")
                        nc.scalar.activation(ysb[:], ps[:], AF.Silu,
                                             bias=b2_t[:, ob:ob + 1])
                        x2 = xp_pool.tile([128, NBLK], f32, tag=f"x{ob}",
                                          name=f"x2_{nb}_{ob}")
                        nc.sync.dma_start(x2[:], x_d[bass.ts(ob, 128), ns])
                        res = work.tile([128, NBLK], f32, tag="wk",
                                        name=f"res{nb}_{ob}")
                        nc.vector.tensor_add(res[:], ysb[:], x2[:])
                        nc.sync.dma_start(y_d[bass.ts(ob, 128), ns], res[:])

    nc.compile()
    return nc


def prep_inputs(inputs):
    """Host-side folding of BN + weight layouts. Returns the shared in_map."""
    i = {k: np.asarray(v, dtype=np.float32) if np.asarray(v).dtype == np.float32
         else np.asarray(v) for k, v in inputs.items()}
    s1 = i["bn1_g"] / np.sqrt(i["bn1_v"] + EPS)
    cv1w = i["cv1_w"] * s1[:, None]                       # [C, DIMS]
    b1 = i["bn1_b"] - i["bn1_m"] * s1                     # [C]
    s2 = i["bn2_g"] / np.sqrt(i["bn2_v"] + EPS)
    cv2w = i["cv2_w"] * s2[:, None]                       # [DIMS, C]
    b2 = i["bn2_b"] - i["bn2_m"] * s2                     # [DIMS]

    n_idx = np.arange(N)
    onehot = np.zeros((128, N), np.float32)
    onehot[n_idx // SIZE, n_idx] = 1.0                    # H rows 0:64
    onehot[64 + n_idx % SIZE, n_idx] = 1.0                # W rows 64:128

    return {
        "cv1_lhsT": np.ascontiguousarray(cv1w.T),         # [DIMS, C]
        "b1": np.ascontiguousarray(b1.reshape(2, 128).T),
        "q_lhsT": np.ascontiguousarray(i["q_w"].T),
        "q_bias": np.ascontiguousarray(i["q_b"].reshape(2, 128).T),
        "k_lhsT": np.ascontiguousarray(i["k_w"].T),
        "e_lhsT": np.ascontiguousarray(i["e_w"].T),
        "v_rhs": np.ascontiguousarray(i["v_w"].T),        # [C, C]: v_rhs[ci,c]
        "v_bias_row": np.ascontiguousarray(i["v_b"].reshape(1, C)),
        "rh": np.ascontiguousarray(i["rel_h"].reshape(C, SIZE)),
        "rw": np.ascontiguousarray(i["rel_w"].reshape(C, SIZE)),
        "onehot": onehot.astype(ml_dtypes.bfloat16),
        "cv2_lhsT": np.ascontiguousarray(cv2w.T),         # [C, DIMS]
        "b2": np.ascontiguousarray(b2.reshape(4, 128).T),
        "ones_col": np.ones((128, 1), np.float32),
        "ones_row": np.ones((1, 128), np.float32),
    }


_NC = None


def run(inputs, trace=False):
    global _NC
    if _NC is None:
        _NC = build_nc()
    shared = prep_inputs(inputs)
    x = np.asarray(inputs["x"], dtype=np.float32)  # [B, DIMS, SIZE, SIZE]
    in_maps = []
    for b in range(B):
        m = dict(shared)
        m["x"] = np.ascontiguousarray(x[b].reshape(DIMS, N))
        in_maps.append(m)
    res = run_bass_kernel_spmd(_NC, in_maps, list(range(B)), trace=trace)
    out = np.stack([res.results[b]["y"].reshape(DIMS, SIZE, SIZE)
                    for b in range(B)], axis=0)
    return out.astype(np.float32), res.exec_time_ns


def kernel(**inputs) -> np.ndarray:
    out, _ = run(inputs, trace=False)
    return out
